# revision 3
# baseline (speedup 1.0000x reference)
"""Causal MHA (B=2, N=2048, D=1024, H=16) on 8 NeuronCores via Bass/Tile.

Sharding: core c = (b, g): b = c // 4 (batch), g = c % 4 (head group of 4
heads = 256 features). Each core computes its Q/K/V projections, causal
attention for its 4 heads, and a partial output projection (its 256 rows of
Wo). The host sums the 4 partials per batch and adds bo. All-bf16 datapath
(fp8 measured 6.7% rel err vs the 2e-2 budget; bf16 lands ~0.5%).

v4 structure, driven by engine-queue discipline (all hw queues are in-order):
- ACT runs ONLY the softmax exp stream (the attention-phase bottleneck);
  every PSUM evacuation moved off it: K/Q evac is a DVE tensor_scalar add
  with the bias as a per-partition scalar AP, V keeps its DVE bv_rep add,
  O-proj evacs are DVE copies.
- Causal tri-masks run on the otherwise idle Pool engine so they never queue
  behind bulk DVE work on the exp -> mask -> P@V critical path.
- The kt loop is software-pipelined (L=4): P@V for tile kt-4 issues after
  scores for tile kt, hiding the exp+mask chain and the end-of-ft normalize
  (otp WAR) behind later scores.
- Projection chains for chunk ss+1 and the deferred O-proj of ss-1 are
  interleaved INTO the attention kt loop as paced PE filler, overlapping the
  PE-bound projection phase with the ACT-bound attention phase instead of
  running them back to back.
- x for chunk ss+1 prefetches during chunk ss; per-(head, q) normalization
  broadcasts the reciprocal row via a DRAM bounce on the ACT DGE queue
  (chunks 0-2) or a K=1 PE broadcast + ACT copy on the last chunk so the
  final O-proj is not gated on a DMA roundtrip.
"""

import numpy as np
import ml_dtypes

import concourse.bass as bass
import concourse.bacc as bacc
import concourse.mybir as mybir
from concourse.tile import TileContext
from concourse.bass_utils import run_bass_kernel_spmd

F32 = mybir.dt.float32
BF16 = mybir.dt.bfloat16
AF = mybir.ActivationFunctionType
NPBF = ml_dtypes.bfloat16

B, N, D, H, DH = 2, 2048, 1024, 16, 64
NCORES = 8
GROUPS = 4
HPC = H // GROUPS     # 4 heads per core
FS = HPC * DH         # 256
P = 128
NDT = N // 16 // 8    # unused; keep namespace tidy
NSS = N // 512        # 4
DT = D // 128         # 8
FT = FS // 128        # 2
NKT = N // 128        # 16
LOOKAHEAD = 4

_CACHE = {}


def _build(repeat=1, phases="all"):
    nc = bacc.Bacc("TRN2", target_bir_lowering=False, debug=False)

    xqT = nc.dram_tensor("xqT", [D, N], BF16, kind="ExternalInput")
    xkvT = nc.dram_tensor("xkvT", [D, N], BF16, kind="ExternalInput")
    wq = nc.dram_tensor("wq", [D, FS], BF16, kind="ExternalInput")
    wk = nc.dram_tensor("wk", [D, FS], BF16, kind="ExternalInput")
    wv = nc.dram_tensor("wv", [D, FS], BF16, kind="ExternalInput")
    wo = nc.dram_tensor("wo", [FS, D], BF16, kind="ExternalInput")
    bq = nc.dram_tensor("bq", [FS], F32, kind="ExternalInput")
    bk = nc.dram_tensor("bk", [FS], F32, kind="ExternalInput")
    bv = nc.dram_tensor("bv", [1, FS], BF16, kind="ExternalInput")
    masks = nc.dram_tensor("masks", [P, 2, P], BF16, kind="ExternalInput")
    out = nc.dram_tensor("out_p", [N, D], BF16, kind="ExternalOutput")

    with TileContext(nc) as tc:
        with (
            tc.tile_pool(name="const", bufs=1) as cp,
            tc.tile_pool(name="xt", bufs=2) as xp,
            tc.tile_pool(name="acts", bufs=1) as ap_,
            tc.tile_pool(name="ps", bufs=2, space="PSUM") as psp,
            tc.tile_pool(name="pt", bufs=6) as ptp,
            tc.tile_pool(name="small", bufs=4) as smp,
            tc.tile_pool(name="osb", bufs=3) as osp,
            tc.tile_pool(name="dsc", bufs=4, space="DRAM") as dsp,
        ):
            wq_sb = cp.tile([P, DT, FS], BF16, tag="wq")
            wk_sb = cp.tile([P, DT, FS], BF16, tag="wk")
            wv_sb = cp.tile([P, DT, FS], BF16, tag="wv")
            wo_sb = cp.tile([P, FT, D], BF16, tag="wo")
            bqk_sb = cp.tile([P, 2, 2], F32, tag="bqk")
            bv_sb = cp.tile([1, FS], BF16, tag="bv")
            tri_sb = cp.tile([P, 2, P], BF16, tag="mask")
            ones_r = cp.tile([1, P], BF16, tag="ones")
            ones_f = cp.tile([P, HPC], BF16, tag="ones_f")
            bv_rep = cp.tile([P, FS], F32, tag="bv_rep")

            # small constants first (the bv replication gates the PE queue)
            nc.scalar.dma_start(out=bv_sb, in_=bv.ap())
            nc.scalar.dma_start(out=bqk_sb[:, 0, :], in_=bk.ap().rearrange("(t p) -> p t", p=P))
            nc.scalar.dma_start(out=bqk_sb[:, 1, :], in_=bq.ap().rearrange("(t p) -> p t", p=P))
            nc.scalar.dma_start(out=tri_sb, in_=masks.ap())
            nc.sync.dma_start(out=wk_sb, in_=wk.ap().rearrange("(t p) f -> p t f", p=P))
            nc.scalar.dma_start(out=wv_sb, in_=wv.ap().rearrange("(t p) f -> p t f", p=P))
            nc.scalar.dma_start(out=wq_sb, in_=wq.ap().rearrange("(t p) f -> p t f", p=P))
            nc.scalar.dma_start(out=wo_sb, in_=wo.ap().rearrange("(t p) f -> p t f", p=P))
            nc.vector.memset(ones_f, 1.0)
            nc.vector.memset(ones_r, 1.0)

            ps_rep = psp.tile([P, 512], F32, tag="ps", name="ps_brep")
            nc.tensor.matmul(ps_rep[:, 0:FS], ones_r[:, 0:P], bv_sb, start=True, stop=True)
            nc.vector.tensor_copy(bv_rep, ps_rep[:, 0:FS])

            kt_all = [ap_.tile([P, N], BF16, tag=f"kt{f}", name=f"kt{f}") for f in range(FT)]
            qt_all = [ap_.tile([P, N], BF16, tag=f"qt{f}", name=f"qt{f}") for f in range(FT)]
            v_sb = [ap_.tile([P, HPC, DH + 1], BF16, tag=f"v{st}", name=f"v{st}") for st in range(NKT)]
            ot_all = [ap_.tile([P, N], BF16, tag=f"ot{f}", name=f"ot{f}") for f in range(FT)]

            for st in range(NKT):
                nc.vector.tensor_copy(v_sb[st][:, :, DH], ones_f)

            def fetch_chunk(ss):
                s0 = ss * 512
                xkv_t, xq_t = [], []
                for d in range(DT):
                    t = xp.tile([P, 512], BF16, tag=f"xkv{d}", name=f"xkv{d}")
                    nc.sync.dma_start(out=t, in_=xkvT.ap()[d * P:(d + 1) * P, s0:s0 + 512])
                    xkv_t.append(t)
                for d in range(DT):
                    t = xp.tile([P, 512], BF16, tag=f"xq{d}", name=f"xq{d}")
                    nc.sync.dma_start(out=t, in_=xqT.ap()[d * P:(d + 1) * P, s0:s0 + 512])
                    xq_t.append(t)
                return xkv_t, xq_t

            def kq_chain(ss, ft, which, xt):
                s0 = ss * 512
                w_sb = wk_sb if which == 0 else wq_sb
                dstt = kt_all[ft] if which == 0 else qt_all[ft]
                ps = psp.tile([P, 512], F32, tag="ps", name="ps_kq")
                for d in range(DT):
                    nc.tensor.matmul(
                        ps, w_sb[:, d, ft * P:(ft + 1) * P], xt[d],
                        start=(d == 0), stop=(d == DT - 1),
                    )
                # DVE evac with per-partition bias column (keeps ACT exp-only)
                nc.vector.tensor_scalar(
                    dstt[:, s0:s0 + 512], ps, bqk_sb[:, which, ft:ft + 1], None,
                    mybir.AluOpType.add,
                )

            def v_chain(ss, st, xt):
                psv = psp.tile([P, 512], F32, tag="ps", name="ps_v")
                for d in range(DT):
                    nc.tensor.matmul(
                        psv[:, 0:FS], xt[d][:, st * P:(st + 1) * P], wv_sb[:, d, :],
                        start=(d == 0), stop=(d == DT - 1),
                    )
                v = v_sb[ss * 4 + st]
                nc.vector.tensor_add(
                    v[:, :, 0:DH],
                    psv[:, 0:FS].rearrange("p (h c) -> p h c", h=HPC),
                    bv_rep.rearrange("p (h c) -> p h c", h=HPC),
                )

            def proj_tasks(ss, tiles):
                xkv_t, xq_t = tiles
                tasks = [lambda ft=ft: kq_chain(ss, ft, 0, xkv_t) for ft in range(FT)]
                tasks += [lambda st=st: v_chain(ss, st, xkv_t) for st in range(4)]
                tasks += [lambda ft=ft: kq_chain(ss, ft, 1, xq_t) for ft in range(FT)]
                return tasks

            def oproj_qt(qt):
                o_sb = osp.tile([P, D], BF16, tag="osb", name="o_sb")
                for os_ in range(2):
                    ps_o = psp.tile([P, 512], F32, tag="ps", name="ps_o")
                    for ft in range(FT):
                        nc.tensor.matmul(
                            ps_o,
                            ot_all[ft][:, qt * P:(qt + 1) * P],
                            wo_sb[:, ft, os_ * 512:(os_ + 1) * 512],
                            start=(ft == 0), stop=(ft == FT - 1),
                        )
                    nc.vector.tensor_copy(o_sb[:, os_ * 512:(os_ + 1) * 512], ps_o)
                nc.sync.dma_start(out=out.ap()[qt * P:(qt + 1) * P, :], in_=o_sb)

            def oproj_tasks(ss):
                return [lambda qt=qt: oproj_qt(qt) for qt in range(4 * ss, 4 * ss + 4)]

            def attention(ss, filler):
                """Attention for chunk ss; pops filler tasks pacing them
                across the kt iterations of both fts."""
                s0 = ss * 512
                n_kt = 4 * ss + 4
                iters_total = 2 * (n_kt + LOOKAHEAD)
                # reserve tasks to bridge the two end-of-ft normalize chains;
                # pace the rest evenly across the kt iterations
                reserve = min(6, len(filler))
                state = {"acc": 0.0, "rate": (len(filler) - reserve) / max(iters_total, 1)}

                def pace():
                    state["acc"] += state["rate"]
                    while filler and len(filler) > reserve and state["acc"] >= 1.0:
                        state["acc"] -= 1.0
                        filler.popleft()()

                def ft_end():
                    nonlocal reserve
                    take = min(3, len(filler))
                    reserve = max(0, reserve - take)
                    for _ in range(take):
                        filler.popleft()()

                for ft in range(FT):
                    otp = [
                        psp.tile([P, 512], F32, tag=f"otp{hh}", bufs=1, name=f"ps_ot{hh}")
                        for hh in range(2)
                    ]
                    ptts = {}

                    def emit_scores_exp(kt):
                        st2 = psp.tile([P, 1024], F32, tag="ps2", bufs=2, name="ps_st2")
                        ptt = ptp.tile([P, 1024], BF16, tag="pt", name="ptt")
                        ptts[kt] = ptt
                        dk = max((kt - 4 * ss) * P, 0)
                        for hh in range(2):
                            nc.tensor.matmul(
                                st2[:, hh * 512 + dk:(hh + 1) * 512],
                                kt_all[ft][hh * 64:(hh + 1) * 64, kt * P:(kt + 1) * P],
                                qt_all[ft][hh * 64:(hh + 1) * 64, s0 + dk:s0 + 512],
                                start=True, stop=True,
                            )
                        if dk == 0:
                            nc.scalar.activation(ptt, st2, AF.Exp, scale=0.125)
                        else:
                            st2v = st2.rearrange("p (h q) -> p h q", h=2)
                            pttv = ptt.rearrange("p (h q) -> p h q", h=2)
                            nc.scalar.activation(
                                pttv[:, :, dk:512], st2v[:, :, dk:512],
                                AF.Exp, scale=0.125,
                            )
                        if kt >= 4 * ss:
                            # causal tri mask on the idle Pool engine: never
                            # queues behind DVE bulk work on the exp->PV path
                            pttv = ptt.rearrange("p (h q) -> p h q", h=2)
                            nc.gpsimd.tensor_tensor(
                                pttv[:, :, dk:dk + P], pttv[:, :, dk:dk + P],
                                tri_sb, mybir.AluOpType.mult,
                            )

                    def emit_pv(kt):
                        ptt = ptts.pop(kt)
                        dk = max((kt - 4 * ss) * P, 0)
                        for hh in range(2):
                            nc.tensor.matmul(
                                otp[hh][0:DH + 1, dk:512],
                                v_sb[kt][:, ft * 2 + hh, :],
                                ptt[:, hh * 512 + dk:(hh + 1) * 512],
                                start=(kt == 0), stop=(kt == n_kt - 1),
                                skip_group_check=True,
                            )

                    for kt in range(n_kt + LOOKAHEAD):
                        if kt < n_kt:
                            emit_scores_exp(kt)
                        if kt >= LOOKAHEAD:
                            emit_pv(kt - LOOKAHEAD)
                        pace()

                    # normalization
                    rept = smp.tile([DH + 1, 1024], F32, tag="rep_sb", bufs=4, name="rept")
                    rep_sb = rept[0:DH, :]
                    if ss == NSS - 1:
                        recip = smp.tile([1, 1024], BF16, tag="recip_b", bufs=2, name="recip_b")
                    else:
                        recip = rept[DH:DH + 1, :]
                    with nc.allow_low_precision(reason="softmax reciprocal"):
                        nc.vector.reciprocal(recip[:, 0:512], otp[0][DH:DH + 1, :])
                        nc.vector.reciprocal(recip[:, 512:1024], otp[1][DH:DH + 1, :])
                    if ss == NSS - 1:
                        # PE K=1 broadcast + ACT copy: short chain, no DMA
                        for hh in range(2):
                            rep_ps = psp.tile([P, 512], F32, tag="ps", name="rep_ps")
                            nc.tensor.matmul(
                                rep_ps[0:DH, :], ones_r[:, 0:DH],
                                recip[:, hh * 512:(hh + 1) * 512],
                                start=True, stop=True,
                            )
                            nc.scalar.copy(rep_sb[:, hh * 512:(hh + 1) * 512], rep_ps[0:DH, :])
                            nc.vector.tensor_mul(
                                ot_all[ft][hh * 64:hh * 64 + 64, s0:s0 + 512],
                                otp[hh][0:DH, :],
                                rep_sb[:, hh * 512:(hh + 1) * 512],
                            )
                    else:
                        dscr = dsp.tile([1, 1024], F32, tag="dscr", name="dscr")
                        nc.scalar.dma_start(out=dscr, in_=recip)
                        rep_bcast = bass.AP(
                            tensor=dscr.tensor,
                            offset=dscr.offset,
                            ap=[[0, DH]] + [list(x) for x in dscr.ap[1:]],
                        )
                        nc.scalar.dma_start(out=rep_sb, in_=rep_bcast)
                        for hh in range(2):
                            nc.vector.tensor_mul(
                                ot_all[ft][hh * 64:hh * 64 + 64, s0:s0 + 512],
                                otp[hh][0:DH, :],
                                rep_sb[:, hh * 512:(hh + 1) * 512],
                            )
                    ft_end()

            from collections import deque

            def emit_body():
                nxt = fetch_chunk(0)
                for ss in range(NSS):
                    tiles = nxt
                    if ss == 0:
                        for t in proj_tasks(0, tiles):
                            t()
                    if ss + 1 < NSS:
                        nxt = fetch_chunk(ss + 1)

                    if phases == "proj":
                        if ss > 0:
                            for t in proj_tasks(ss, tiles):
                                t()
                        continue

                    filler = deque()
                    if ss > 0:
                        filler.extend(oproj_tasks(ss - 1))
                    if ss + 1 < NSS:
                        filler.extend(proj_tasks(ss + 1, nxt))

                    attention(ss, filler)
                    while filler:
                        filler.popleft()()
                    if phases != "proj+attn" and ss == NSS - 1:
                        for t in oproj_tasks(ss):
                            t()

                if phases == "proj":
                    row = 0
                    for tset in (kt_all, qt_all):
                        for tt in tset:
                            for half in range(2):
                                nc.sync.dma_start(
                                    out=out.ap()[row * P:(row + 1) * P, :],
                                    in_=tt[:, half * D:(half + 1) * D],
                                )
                                row += 1
                    for st in range(NKT):
                        rr = 8 + st % 8
                        nc.sync.dma_start(
                            out=out.ap()[rr * P:(rr + 1) * P, 0:HPC * (DH + 1)],
                            in_=v_sb[st].rearrange("p h c -> p (h c)"),
                        )
                elif phases == "proj+attn":
                    row = 0
                    for tt in ot_all:
                        for half in range(2):
                            nc.sync.dma_start(
                                out=out.ap()[row * P:(row + 1) * P, :],
                                in_=tt[:, half * D:(half + 1) * D],
                            )
                            row += 1

            if repeat == 1:
                emit_body()
            else:
                with tc.For_i(0, repeat, 1):
                    emit_body()

    nc.compile()
    return nc


def _shard_inputs(x_q, x_kv, Wq, bq_, Wk, bk_, Wv, bv_, Wo, bo_):
    pp_, ff = np.meshgrid(np.arange(P), np.arange(P), indexing="ij")
    tri1 = (ff >= pp_).astype(NPBF)
    tri = np.ascontiguousarray(np.stack([tri1, tri1], axis=1))  # [128, 2, 128]
    in_maps = []
    for c in range(NCORES):
        b, g = c // GROUPS, c % GROUPS
        sl = slice(g * FS, (g + 1) * FS)
        in_maps.append({
            "xqT": np.ascontiguousarray(x_q[b].T).astype(NPBF),
            "xkvT": np.ascontiguousarray(x_kv[b].T).astype(NPBF),
            "wq": np.ascontiguousarray(Wq[:, sl]).astype(NPBF),
            "wk": np.ascontiguousarray(Wk[:, sl]).astype(NPBF),
            "wv": np.ascontiguousarray(Wv[:, sl]).astype(NPBF),
            "wo": np.ascontiguousarray(Wo[sl, :]).astype(NPBF),
            "bq": np.ascontiguousarray(bq_[sl]),
            "bk": np.ascontiguousarray(bk_[sl]),
            "bv": np.ascontiguousarray(bv_[sl]).reshape(1, FS).astype(NPBF),
            "masks": tri,
        })
    return in_maps


def kernel(x_q, x_kv, Wq, bq, Wk, bk, Wv, bv, Wo, bo):
    x_q = np.asarray(x_q, dtype=np.float32)
    x_kv = np.asarray(x_kv, dtype=np.float32)
    if "nc" not in _CACHE:
        _CACHE["nc"] = _build()
    nc = _CACHE["nc"]
    in_maps = _shard_inputs(
        x_q, x_kv,
        np.asarray(Wq, np.float32), np.asarray(bq, np.float32),
        np.asarray(Wk, np.float32), np.asarray(bk, np.float32),
        np.asarray(Wv, np.float32), np.asarray(bv, np.float32),
        np.asarray(Wo, np.float32), np.asarray(bo, np.float32),
    )
    res = run_bass_kernel_spmd(nc, in_maps, core_ids=list(range(NCORES)))
    out = np.zeros((B, N, D), dtype=np.float32)
    for c in range(NCORES):
        out[c // GROUPS] += np.asarray(res.results[c]["out_p"], dtype=np.float32)
    out += np.asarray(bo, np.float32)[None, None, :]
    return out


# revision 4
# speedup vs baseline: 1.0099x; 1.0099x over previous
"""Causal MHA (B=2, N=2048, D=1024, H=16) on 8 NeuronCores via Bass/Tile.

Sharding: core c = (b, g): b = c // 4 (batch), g = c % 4 (head group of 4
heads = 256 features). Each core computes its Q/K/V projections, causal
attention for its 4 heads, and a partial output projection (its 256 rows of
Wo). The host sums the 4 partials per batch and adds bo. All-bf16 datapath
(fp8 measured 6.7% rel err vs the 2e-2 budget; bf16 lands ~0.5%).

v4 structure, driven by engine-queue discipline (all hw queues are in-order):
- ACT runs ONLY the softmax exp stream (the attention-phase bottleneck);
  every PSUM evacuation moved off it: K/Q evac is a DVE tensor_scalar add
  with the bias as a per-partition scalar AP, V keeps its DVE bv_rep add,
  O-proj evacs are DVE copies.
- Causal tri-masks run on the otherwise idle Pool engine so they never queue
  behind bulk DVE work on the exp -> mask -> P@V critical path.
- The kt loop is software-pipelined (L=4): P@V for tile kt-4 issues after
  scores for tile kt, hiding the exp+mask chain and the end-of-ft normalize
  (otp WAR) behind later scores.
- Projection chains for chunk ss+1 and the deferred O-proj of ss-1 are
  interleaved INTO the attention kt loop as paced PE filler at d-pair
  micro-task granularity (~0.5us each, one open PSUM chain at a time within
  the 2-buffer rotation), overlapping the PE-bound projection phase with the
  ACT-bound attention phase instead of running them back to back.
- x for chunk ss+1 prefetches during chunk ss; per-(head, q) normalization
  broadcasts the reciprocal row via a DRAM bounce on the ACT DGE queue
  (chunks 0-2) or a K=1 PE broadcast + ACT copy on the last chunk so the
  final O-proj is not gated on a DMA roundtrip.
"""

import numpy as np
import ml_dtypes

import concourse.bass as bass
import concourse.bacc as bacc
import concourse.mybir as mybir
from concourse.tile import TileContext
from concourse.bass_utils import run_bass_kernel_spmd

F32 = mybir.dt.float32
BF16 = mybir.dt.bfloat16
AF = mybir.ActivationFunctionType
NPBF = ml_dtypes.bfloat16

B, N, D, H, DH = 2, 2048, 1024, 16, 64
NCORES = 8
GROUPS = 4
HPC = H // GROUPS     # 4 heads per core
FS = HPC * DH         # 256
P = 128
NDT = N // 16 // 8    # unused; keep namespace tidy
NSS = N // 512        # 4
DT = D // 128         # 8
FT = FS // 128        # 2
NKT = N // 128        # 16
LOOKAHEAD = 4

_CACHE = {}


def _build(repeat=1, phases="all"):
    nc = bacc.Bacc("TRN2", target_bir_lowering=False, debug=False)

    xqT = nc.dram_tensor("xqT", [D, N], BF16, kind="ExternalInput")
    xkvT = nc.dram_tensor("xkvT", [D, N], BF16, kind="ExternalInput")
    wq = nc.dram_tensor("wq", [D, FS], BF16, kind="ExternalInput")
    wk = nc.dram_tensor("wk", [D, FS], BF16, kind="ExternalInput")
    wv = nc.dram_tensor("wv", [D, FS], BF16, kind="ExternalInput")
    wo = nc.dram_tensor("wo", [FS, D], BF16, kind="ExternalInput")
    bq = nc.dram_tensor("bq", [FS], F32, kind="ExternalInput")
    bk = nc.dram_tensor("bk", [FS], F32, kind="ExternalInput")
    bv = nc.dram_tensor("bv", [1, FS], BF16, kind="ExternalInput")
    masks = nc.dram_tensor("masks", [P, 2, P], BF16, kind="ExternalInput")
    out = nc.dram_tensor("out_p", [N, D], BF16, kind="ExternalOutput")

    with TileContext(nc) as tc:
        with (
            tc.tile_pool(name="const", bufs=1) as cp,
            tc.tile_pool(name="xt", bufs=2) as xp,
            tc.tile_pool(name="acts", bufs=1) as ap_,
            tc.tile_pool(name="ps", bufs=2, space="PSUM") as psp,
            tc.tile_pool(name="pt", bufs=6) as ptp,
            tc.tile_pool(name="small", bufs=4) as smp,
            tc.tile_pool(name="osb", bufs=3) as osp,
            tc.tile_pool(name="dsc", bufs=4, space="DRAM") as dsp,
        ):
            wq_sb = cp.tile([P, DT, FS], BF16, tag="wq")
            wk_sb = cp.tile([P, DT, FS], BF16, tag="wk")
            wv_sb = cp.tile([P, DT, FS], BF16, tag="wv")
            wo_sb = cp.tile([P, FT, D], BF16, tag="wo")
            bqk_sb = cp.tile([P, 2, 2], F32, tag="bqk")
            bv_sb = cp.tile([1, FS], BF16, tag="bv")
            tri_sb = cp.tile([P, 2, P], BF16, tag="mask")
            ones_r = cp.tile([1, P], BF16, tag="ones")
            ones_f = cp.tile([P, HPC], BF16, tag="ones_f")
            bv_rep = cp.tile([P, FS], F32, tag="bv_rep")

            # small constants first (the bv replication gates the PE queue)
            nc.scalar.dma_start(out=bv_sb, in_=bv.ap())
            nc.scalar.dma_start(out=bqk_sb[:, 0, :], in_=bk.ap().rearrange("(t p) -> p t", p=P))
            nc.scalar.dma_start(out=bqk_sb[:, 1, :], in_=bq.ap().rearrange("(t p) -> p t", p=P))
            nc.scalar.dma_start(out=tri_sb, in_=masks.ap())
            nc.sync.dma_start(out=wk_sb, in_=wk.ap().rearrange("(t p) f -> p t f", p=P))
            nc.scalar.dma_start(out=wv_sb, in_=wv.ap().rearrange("(t p) f -> p t f", p=P))
            nc.scalar.dma_start(out=wq_sb, in_=wq.ap().rearrange("(t p) f -> p t f", p=P))
            nc.scalar.dma_start(out=wo_sb, in_=wo.ap().rearrange("(t p) f -> p t f", p=P))
            nc.vector.memset(ones_f, 1.0)
            nc.vector.memset(ones_r, 1.0)

            ps_rep = psp.tile([P, 512], F32, tag="ps", name="ps_brep")
            nc.tensor.matmul(ps_rep[:, 0:FS], ones_r[:, 0:P], bv_sb, start=True, stop=True)
            nc.vector.tensor_copy(bv_rep, ps_rep[:, 0:FS])

            kt_all = [ap_.tile([P, N], BF16, tag=f"kt{f}", name=f"kt{f}") for f in range(FT)]
            qt_all = [ap_.tile([P, N], BF16, tag=f"qt{f}", name=f"qt{f}") for f in range(FT)]
            v_sb = [ap_.tile([P, HPC, DH + 1], BF16, tag=f"v{st}", name=f"v{st}") for st in range(NKT)]
            ot_all = [ap_.tile([P, N], BF16, tag=f"ot{f}", name=f"ot{f}") for f in range(FT)]

            for st in range(NKT):
                nc.vector.tensor_copy(v_sb[st][:, :, DH], ones_f)

            def fetch_chunk(ss):
                s0 = ss * 512
                xkvr = xkvT.ap().rearrange("(t p) n -> p t n", p=P)
                xqr = xqT.ap().rearrange("(t p) n -> p t n", p=P)
                xkv_t, xq_t = [], []
                for j in range(DT // 2):
                    t = xp.tile([P, 2, 512], BF16, tag=f"xkv{j}", name=f"xkv{j}")
                    nc.sync.dma_start(out=t, in_=xkvr[:, 2 * j:2 * j + 2, s0:s0 + 512])
                    xkv_t += [t[:, 0, :], t[:, 1, :]]
                for j in range(DT // 2):
                    t = xp.tile([P, 2, 512], BF16, tag=f"xq{j}", name=f"xq{j}")
                    nc.sync.dma_start(out=t, in_=xqr[:, 2 * j:2 * j + 2, s0:s0 + 512])
                    xq_t += [t[:, 0, :], t[:, 1, :]]
                return xkv_t, xq_t

            def kq_micro(ss, ft, which, xt):
                # one PSUM chain split into d-pair micro-tasks; the chain's
                # psum tile is shared across the closures via `box`
                s0 = ss * 512
                w_sb = wk_sb if which == 0 else wq_sb
                dstt = kt_all[ft] if which == 0 else qt_all[ft]
                box = {}

                def step(dp):
                    if dp == 0:
                        box["ps"] = psp.tile([P, 512], F32, tag="ps", name="ps_kq")
                    ps = box["ps"]
                    for d in (2 * dp, 2 * dp + 1):
                        nc.tensor.matmul(
                            ps, w_sb[:, d, ft * P:(ft + 1) * P], xt[d],
                            start=(d == 0), stop=(d == DT - 1),
                        )
                    if dp == DT // 2 - 1:
                        # DVE evac, per-partition bias column (ACT stays exp-only)
                        nc.vector.tensor_scalar(
                            dstt[:, s0:s0 + 512], ps, bqk_sb[:, which, ft:ft + 1],
                            None, mybir.AluOpType.add,
                        )
                return [lambda dp=dp: step(dp) for dp in range(DT // 2)]

            def v_micro(ss, st, xt):
                box = {}

                def step(dp):
                    if dp == 0:
                        box["ps"] = psp.tile([P, 512], F32, tag="ps", name="ps_v")
                    psv = box["ps"]
                    for d in (2 * dp, 2 * dp + 1):
                        nc.tensor.matmul(
                            psv[:, 0:FS], xt[d][:, st * P:(st + 1) * P], wv_sb[:, d, :],
                            start=(d == 0), stop=(d == DT - 1),
                        )
                    if dp == DT // 2 - 1:
                        v = v_sb[ss * 4 + st]
                        nc.vector.tensor_add(
                            v[:, :, 0:DH],
                            psv[:, 0:FS].rearrange("p (h c) -> p h c", h=HPC),
                            bv_rep.rearrange("p (h c) -> p h c", h=HPC),
                        )
                return [lambda dp=dp: step(dp) for dp in range(DT // 2)]

            def proj_tasks(ss, tiles):
                xkv_t, xq_t = tiles
                tasks = []
                for ft in range(FT):
                    tasks += kq_micro(ss, ft, 0, xkv_t)
                for st in range(4):
                    tasks += v_micro(ss, st, xkv_t)
                for ft in range(FT):
                    tasks += kq_micro(ss, ft, 1, xq_t)
                return tasks

            def oproj_qt_micro(qt):
                box = {}

                def step(k):
                    os_ = k // 2
                    ft = k % 2
                    if k == 0:
                        box["o_sb"] = osp.tile([P, D], BF16, tag="osb", name="o_sb")
                    if ft == 0:
                        box["ps"] = psp.tile([P, 512], F32, tag="ps", name="ps_o")
                    ps_o = box["ps"]
                    nc.tensor.matmul(
                        ps_o,
                        ot_all[ft][:, qt * P:(qt + 1) * P],
                        wo_sb[:, ft, os_ * 512:(os_ + 1) * 512],
                        start=(ft == 0), stop=(ft == FT - 1),
                    )
                    if ft == FT - 1:
                        nc.vector.tensor_copy(
                            box["o_sb"][:, os_ * 512:(os_ + 1) * 512], ps_o)
                    if k == 3:
                        nc.sync.dma_start(
                            out=out.ap()[qt * P:(qt + 1) * P, :], in_=box["o_sb"])
                return [lambda k=k: step(k) for k in range(4)]

            def oproj_tasks(ss):
                tasks = []
                for qt in range(4 * ss, 4 * ss + 4):
                    tasks += oproj_qt_micro(qt)
                return tasks

            def attention(ss, filler):
                """Attention for chunk ss; pops filler tasks pacing them
                across the kt iterations of both fts."""
                s0 = ss * 512
                n_kt = 4 * ss + 4
                iters_total = 2 * (n_kt + LOOKAHEAD)
                # reserve tasks to bridge the two end-of-ft normalize chains;
                # pace the rest evenly across the kt iterations
                reserve = min(8, len(filler))
                state = {"acc": 0.0, "rate": (len(filler) - reserve) / max(iters_total, 1)}

                def pace():
                    state["acc"] += state["rate"]
                    while filler and len(filler) > reserve and state["acc"] >= 1.0:
                        state["acc"] -= 1.0
                        filler.popleft()()

                def ft_end():
                    nonlocal reserve
                    take = min(4, len(filler))
                    reserve = max(0, reserve - take)
                    for _ in range(take):
                        filler.popleft()()

                for ft in range(FT):
                    otp = [
                        psp.tile([P, 512], F32, tag=f"otp{hh}", bufs=1, name=f"ps_ot{hh}")
                        for hh in range(2)
                    ]
                    ptts = {}

                    def emit_scores_exp(kt):
                        st2 = psp.tile([P, 1024], F32, tag="ps2", bufs=2, name="ps_st2")
                        ptt = ptp.tile([P, 1024], BF16, tag="pt", name="ptt")
                        ptts[kt] = ptt
                        dk = max((kt - 4 * ss) * P, 0)
                        for hh in range(2):
                            nc.tensor.matmul(
                                st2[:, hh * 512 + dk:(hh + 1) * 512],
                                kt_all[ft][hh * 64:(hh + 1) * 64, kt * P:(kt + 1) * P],
                                qt_all[ft][hh * 64:(hh + 1) * 64, s0 + dk:s0 + 512],
                                start=True, stop=True,
                            )
                        if dk == 0:
                            nc.scalar.activation(ptt, st2, AF.Exp, scale=0.125)
                        else:
                            st2v = st2.rearrange("p (h q) -> p h q", h=2)
                            pttv = ptt.rearrange("p (h q) -> p h q", h=2)
                            nc.scalar.activation(
                                pttv[:, :, dk:512], st2v[:, :, dk:512],
                                AF.Exp, scale=0.125,
                            )
                        if kt >= 4 * ss:
                            # causal tri mask on the idle Pool engine: never
                            # queues behind DVE bulk work on the exp->PV path
                            pttv = ptt.rearrange("p (h q) -> p h q", h=2)
                            nc.gpsimd.tensor_tensor(
                                pttv[:, :, dk:dk + P], pttv[:, :, dk:dk + P],
                                tri_sb, mybir.AluOpType.mult,
                            )

                    def emit_pv(kt):
                        ptt = ptts.pop(kt)
                        dk = max((kt - 4 * ss) * P, 0)
                        for hh in range(2):
                            nc.tensor.matmul(
                                otp[hh][0:DH + 1, dk:512],
                                v_sb[kt][:, ft * 2 + hh, :],
                                ptt[:, hh * 512 + dk:(hh + 1) * 512],
                                start=(kt == 0), stop=(kt == n_kt - 1),
                                skip_group_check=True,
                            )

                    for kt in range(n_kt + LOOKAHEAD):
                        if kt < n_kt:
                            emit_scores_exp(kt)
                        if kt >= LOOKAHEAD:
                            emit_pv(kt - LOOKAHEAD)
                        pace()

                    # normalization
                    rept = smp.tile([DH + 1, 1024], F32, tag="rep_sb", bufs=4, name="rept")
                    rep_sb = rept[0:DH, :]
                    if ss == NSS - 1:
                        recip = smp.tile([1, 1024], BF16, tag="recip_b", bufs=2, name="recip_b")
                    else:
                        recip = rept[DH:DH + 1, :]
                    with nc.allow_low_precision(reason="softmax reciprocal"):
                        nc.vector.reciprocal(recip[:, 0:512], otp[0][DH:DH + 1, :])
                        nc.vector.reciprocal(recip[:, 512:1024], otp[1][DH:DH + 1, :])
                    if ss == NSS - 1:
                        # PE K=1 broadcast + ACT copy: short chain, no DMA
                        for hh in range(2):
                            rep_ps = psp.tile([P, 512], F32, tag="ps", name="rep_ps")
                            nc.tensor.matmul(
                                rep_ps[0:DH, :], ones_r[:, 0:DH],
                                recip[:, hh * 512:(hh + 1) * 512],
                                start=True, stop=True,
                            )
                            nc.scalar.copy(rep_sb[:, hh * 512:(hh + 1) * 512], rep_ps[0:DH, :])
                            nc.vector.tensor_mul(
                                ot_all[ft][hh * 64:hh * 64 + 64, s0:s0 + 512],
                                otp[hh][0:DH, :],
                                rep_sb[:, hh * 512:(hh + 1) * 512],
                            )
                    else:
                        dscr = dsp.tile([1, 1024], F32, tag="dscr", name="dscr")
                        nc.scalar.dma_start(out=dscr, in_=recip)
                        rep_bcast = bass.AP(
                            tensor=dscr.tensor,
                            offset=dscr.offset,
                            ap=[[0, DH]] + [list(x) for x in dscr.ap[1:]],
                        )
                        nc.scalar.dma_start(out=rep_sb, in_=rep_bcast)
                        for hh in range(2):
                            nc.vector.tensor_mul(
                                ot_all[ft][hh * 64:hh * 64 + 64, s0:s0 + 512],
                                otp[hh][0:DH, :],
                                rep_sb[:, hh * 512:(hh + 1) * 512],
                            )
                    ft_end()

            from collections import deque

            def emit_body():
                nxt = fetch_chunk(0)
                for ss in range(NSS):
                    tiles = nxt
                    if ss == 0:
                        for t in proj_tasks(0, tiles):
                            t()
                    if ss + 1 < NSS:
                        nxt = fetch_chunk(ss + 1)

                    if phases == "proj":
                        if ss > 0:
                            for t in proj_tasks(ss, tiles):
                                t()
                        continue

                    filler = deque()
                    if ss > 0:
                        filler.extend(oproj_tasks(ss - 1))
                    if ss + 1 < NSS:
                        filler.extend(proj_tasks(ss + 1, nxt))

                    attention(ss, filler)
                    while filler:
                        filler.popleft()()
                    if phases != "proj+attn" and ss == NSS - 1:
                        for t in oproj_tasks(ss):
                            t()

                if phases == "proj":
                    row = 0
                    for tset in (kt_all, qt_all):
                        for tt in tset:
                            for half in range(2):
                                nc.sync.dma_start(
                                    out=out.ap()[row * P:(row + 1) * P, :],
                                    in_=tt[:, half * D:(half + 1) * D],
                                )
                                row += 1
                    for st in range(NKT):
                        rr = 8 + st % 8
                        nc.sync.dma_start(
                            out=out.ap()[rr * P:(rr + 1) * P, 0:HPC * (DH + 1)],
                            in_=v_sb[st].rearrange("p h c -> p (h c)"),
                        )
                elif phases == "proj+attn":
                    row = 0
                    for tt in ot_all:
                        for half in range(2):
                            nc.sync.dma_start(
                                out=out.ap()[row * P:(row + 1) * P, :],
                                in_=tt[:, half * D:(half + 1) * D],
                            )
                            row += 1

            if repeat == 1:
                emit_body()
            else:
                with tc.For_i(0, repeat, 1):
                    emit_body()

    nc.compile()
    return nc


def _shard_inputs(x_q, x_kv, Wq, bq_, Wk, bk_, Wv, bv_, Wo, bo_):
    pp_, ff = np.meshgrid(np.arange(P), np.arange(P), indexing="ij")
    tri1 = (ff >= pp_).astype(NPBF)
    tri = np.ascontiguousarray(np.stack([tri1, tri1], axis=1))  # [128, 2, 128]
    in_maps = []
    for c in range(NCORES):
        b, g = c // GROUPS, c % GROUPS
        sl = slice(g * FS, (g + 1) * FS)
        in_maps.append({
            "xqT": np.ascontiguousarray(x_q[b].T).astype(NPBF),
            "xkvT": np.ascontiguousarray(x_kv[b].T).astype(NPBF),
            "wq": np.ascontiguousarray(Wq[:, sl]).astype(NPBF),
            "wk": np.ascontiguousarray(Wk[:, sl]).astype(NPBF),
            "wv": np.ascontiguousarray(Wv[:, sl]).astype(NPBF),
            "wo": np.ascontiguousarray(Wo[sl, :]).astype(NPBF),
            "bq": np.ascontiguousarray(bq_[sl]),
            "bk": np.ascontiguousarray(bk_[sl]),
            "bv": np.ascontiguousarray(bv_[sl]).reshape(1, FS).astype(NPBF),
            "masks": tri,
        })
    return in_maps


def kernel(x_q, x_kv, Wq, bq, Wk, bk, Wv, bv, Wo, bo):
    x_q = np.asarray(x_q, dtype=np.float32)
    x_kv = np.asarray(x_kv, dtype=np.float32)
    if "nc" not in _CACHE:
        _CACHE["nc"] = _build()
    nc = _CACHE["nc"]
    in_maps = _shard_inputs(
        x_q, x_kv,
        np.asarray(Wq, np.float32), np.asarray(bq, np.float32),
        np.asarray(Wk, np.float32), np.asarray(bk, np.float32),
        np.asarray(Wv, np.float32), np.asarray(bv, np.float32),
        np.asarray(Wo, np.float32), np.asarray(bo, np.float32),
    )
    res = run_bass_kernel_spmd(nc, in_maps, core_ids=list(range(NCORES)))
    out = np.zeros((B, N, D), dtype=np.float32)
    for c in range(NCORES):
        out[c // GROUPS] += np.asarray(res.results[c]["out_p"], dtype=np.float32)
    out += np.asarray(bo, np.float32)[None, None, :]
    return out
